# revision 17
# baseline (speedup 1.0000x reference)
"""2-layer GCN (PyG GCNConv style) on 8 Trainium2 NeuronCores.

Strategy (graph/node parallel, per sharding hint):
  - Nodes are range-sharded across 8 cores (R = N/8 rows each).
  - Host packs each core's incoming edges (incl. self-loops) into 128-edge
    chunks grouped by (source-node half, destination tile); pad slots get
    norm=0. Host also prebuilds the per-chunk selection matrices
    S[e, d] = (dst_local[e]==d) * norm[e] as one bf16 array ("smat"),
    consumed by both layers -- no per-chunk DVE work on device.
  - All PE-path data is bf16 (fp32 matmul is 4 cycles/row vs 1 for bf16;
    bf16 also halves AllGather wire bytes and gather descriptors).
  - Row gathers use the DMAGatherAnt custom instruction, batched up to
    SG=7 chunks (896 descriptors) per instruction: the SWDGE ring holds
    1024 descriptors (hard ucode limit, verified by probe) and descgen
    costs ~1us fixed per instruction. Indices are int16, so the gather
    source is split into NH=2 node-range halves (25000 rows < 32768).
  - Device per core:
      phase 1: xw1 = x_c @ W1 (own rows)           -> AllGather full xw1
      phase 2: stream (gather G rows | load S tiles), PE-matmul-
               accumulate S.T @ G per dst tile, ReLU+bias, PE-transpose
               into h1T (bf16, kept in SBUF)
      phase 3: hw2 = h1 @ W2 (uses h1T as lhsT)    -> AllGather full hw2
      phase 4: same aggregation on hw2, bias via rank-1 ones@b2 matmul,
               fp32 DMA to output.
  - Host concatenates the 8 row-shards.
"""

import sys

for p in ("/opt/trn_rl_repo",):
    if p not in sys.path:
        sys.path.insert(0, p)

import numpy as np
import ml_dtypes

import concourse.bacc as bacc
import concourse.mybir as mybir
import concourse.tile as tile
from concourse import bass_utils
from concourse.masks import make_identity

P = 128
NCORES = 8
NH = 2   # node-range halves for int16 gather indices
SG = 7   # max chunks per dma_gather (896 descs < 1024-desc SWDGE ring;
         # probe-measured best gather throughput at 4 SWDGE queues)
SL = 28  # chunks per S-matrix load (big contiguous HWDGE loads keep the
         # Sync engine off the critical path)
BF16 = ml_dtypes.bfloat16


# ----------------------------------------------------------------------------
# Host-side preprocessing
# ----------------------------------------------------------------------------

def _preprocess(x, edge_index, n_cores):
    """Packs per-core edges grouped by (src half, dst tile).

    Returns per-core (idx16, smat) + chunk table.
      idx16: [128, total*8] int16, dma_gather layout (16-partition
             wrapped, replicated 8x down the partition dim)
      smat:  [128, total*128] bf16, chunk j's S matrix in cols
             [j*128, (j+1)*128)
    """
    N = x.shape[0]
    R = N // n_cores
    assert R * n_cores == N
    ntiles = (R + P - 1) // P
    half = N // NH
    assert half * NH == N and half <= 32767

    src = edge_index[0].astype(np.int64)
    dst = edge_index[1].astype(np.int64)
    loops = np.arange(N, dtype=np.int64)
    src = np.concatenate([src, loops])
    dst = np.concatenate([dst, loops])

    deg = np.bincount(dst, minlength=N).astype(np.float32)
    dis = np.where(deg > 0, 1.0 / np.sqrt(deg), 0.0).astype(np.float32)
    norm = (dis[src] * dis[dst]).astype(np.float32)

    core_id = dst // R
    dloc = dst - core_id * R
    tl = dloc // P
    dstl = (dloc - tl * P).astype(np.int64)
    hf = src // half  # source half of each edge
    grp = hf * ntiles + tl  # per-core group id (half-major, tile-minor)

    ngrp = NH * ntiles
    counts = np.bincount(core_id * ngrp + grp,
                         minlength=n_cores * ngrp).reshape(n_cores, ngrp)
    # per-group chunk count = max over cores (SPMD program is shared)
    chunks = np.ceil(counts.max(axis=0) / P).astype(np.int64)  # [ngrp]
    total = int(chunks.sum())
    offs = np.concatenate([[0], np.cumsum(chunks)])  # chunk offsets per group

    packed = []
    for c in range(n_cores):
        m = core_id == c
        s_c = src[m].astype(np.int32)
        g_c = grp[m]
        d_c = dstl[m]
        n_c = norm[m]
        h_c = hf[m]
        order = np.argsort(g_c, kind="stable")
        s_c, g_c, d_c, n_c, h_c = (s_c[order], g_c[order], d_c[order],
                                   n_c[order], h_c[order])
        cnt = np.bincount(g_c, minlength=ngrp)
        starts = np.cumsum(cnt) - cnt
        pos = np.arange(len(g_c)) - np.repeat(starts, cnt)
        slots = offs[g_c] * P + pos  # slot s = chunk*128 + partition
        A_src = np.zeros(total * P, np.int32)  # half-local source idx
        A_src[slots] = s_c - h_c.astype(np.int32) * half

        # idx16: linear j = chunk*128+p -> [16, J/16] wrapped, replicated
        # 8x down partitions
        idx = A_src.astype(np.int16).reshape(total * P // 16, 16).T
        idx16 = np.ascontiguousarray(np.tile(idx, (8, 1)))  # [128, total*8]

        # smat: [128, total*128] with S_j at cols j*128..(j+1)*128
        smat = np.zeros((P, total * P), np.float32)
        pj = slots % P           # partition (edge slot in chunk)
        cj = slots // P          # global chunk
        smat[pj, cj * P + d_c] = n_c
        packed.append((idx16, smat.astype(BF16)))
    return packed, chunks.tolist(), offs.tolist(), R, ntiles, half


# ----------------------------------------------------------------------------
# Device kernel builder
# ----------------------------------------------------------------------------

def build_nc(N, R, ntiles, chunks, offs, half, F0, F1, F2, n_cores):
    """Build the SPMD Bass program. All dims: F0,F1,F2 multiples of 128."""
    f32 = mybir.dt.float32
    bf16 = mybir.dt.bfloat16
    i16 = mybir.dt.int16
    K0 = F0 // P       # k-tiles in layer-1 matmul
    H1 = F1 // P       # 128-wide halves of F1
    K2 = F1 // P       # k-tiles in layer-2 matmul (= H1)
    assert F2 <= 512 and F2 % P == 0
    last_rows = R - (ntiles - 1) * P  # rows in the final (possibly partial) tile
    RP = ntiles * P    # padded row count
    total = int(sum(chunks))
    S16 = total * P // 16  # idx16 columns

    # super-gather plan: windows of <=SG chunks, never crossing a half
    # boundary (each half reads a different in_ap slice)
    sg_plan = []           # (h, w0, w1) in global chunk coords
    sg_of_chunk = {}       # chunk j -> sg index
    for h in range(NH):
        lo = offs[h * ntiles]
        hi = offs[(h + 1) * ntiles]
        w = lo
        while w < hi:
            w1 = min(w + SG, hi)
            for j in range(w, w1):
                sg_of_chunk[j] = len(sg_plan)
            sg_plan.append((h, w, w1))
            w = w1
    # S-load plan: big contiguous windows over the whole chunk range
    sl_plan = [(w, min(w + SL, total)) for w in range(0, total, SL)]
    sl_of_chunk = {}
    for i, (w0, w1) in enumerate(sl_plan):
        for j in range(w0, w1):
            sl_of_chunk[j] = i

    nc = bacc.Bacc("TRN2", target_bir_lowering=False, debug=False,
                   num_devices=n_cores, num_swdge_queues=4)

    xT = nc.dram_tensor("xT", [F0, R], bf16, kind="ExternalInput").ap()
    idx_d = nc.dram_tensor("idx16", [P, S16], i16, kind="ExternalInput").ap()
    smat_d = nc.dram_tensor("smat", [P, total * P], bf16,
                            kind="ExternalInput").ap()
    W1_d = nc.dram_tensor("W1", [F0, F1], bf16, kind="ExternalInput").ap()
    b1_d = nc.dram_tensor("b1", [F1], bf16, kind="ExternalInput").ap()
    W2_d = nc.dram_tensor("W2", [F1, F2], bf16, kind="ExternalInput").ap()
    b2_d = nc.dram_tensor("b2", [F2], bf16, kind="ExternalInput").ap()
    out_d = nc.dram_tensor("out", [R, F2], f32, kind="ExternalOutput").ap()

    rg = [list(range(n_cores))]

    with tile.TileContext(nc) as tc:
        with (
            tc.tile_pool(name="dram", bufs=1, space="DRAM") as dram,
            tc.tile_pool(name="const", bufs=1) as const,
        ):
            ag1_in = dram.tile([R, F1], bf16)
            ag1_out = dram.tile([N, F1], bf16, addr_space="Shared")
            ag2_in = dram.tile([R, F2], bf16)
            ag2_out = dram.tile([N, F2], bf16, addr_space="Shared")

            # constants (single DMAs with 3D APs to keep consumer wait
            # counts low -- walrus limits sync-waits per instruction)
            w1_sb = const.tile([P, K0 * F1], bf16)
            nc.sync.dma_start(
                out=w1_sb[:].rearrange("p (k f) -> p k f", k=K0),
                in_=W1_d.rearrange("(k p) f -> p k f", p=P))
            w2_sb = const.tile([P, K2 * F2], bf16)
            nc.sync.dma_start(
                out=w2_sb[:].rearrange("p (k f) -> p k f", k=K2),
                in_=W2_d.rearrange("(k p) f -> p k f", p=P))
            b1_row = const.tile([1, F1], bf16)
            nc.sync.dma_start(out=b1_row[:, :], in_=b1_d[None, :])
            b2_row = const.tile([1, F2], bf16)
            nc.sync.dma_start(out=b2_row[:, :], in_=b2_d[None, :])
            ones_col = const.tile([1, P], bf16)
            nc.vector.memset(ones_col[:], 1.0)
            ident = const.tile([P, P], bf16)
            make_identity(nc, ident[:])

            idx_sb = const.tile([P, S16], i16)
            nc.sync.dma_start(out=idx_sb[:], in_=idx_d[:])

            h1T = const.tile([P, H1 * RP], bf16)  # h1 transposed, H1 row-blocks

            # ---------------- phase 1: xw1 = x_c @ W1 ----------------
            with (
                tc.tile_pool(name="p1x", bufs=3) as p1x,
                tc.tile_pool(name="p1o", bufs=3) as p1o,
                tc.tile_pool(name="p1ps", bufs=2, space="PSUM") as p1ps,
            ):
                for m in range(ntiles):
                    rows = last_rows if m == ntiles - 1 else P
                    xt = p1x.tile([P, K0 * P], bf16, tag="xt")
                    nc.sync.dma_start(
                        out=xt[:, : K0 * rows].rearrange(
                            "p (k r) -> p k r", k=K0),
                        in_=xT[:, m * P: m * P + rows].rearrange(
                            "(k p) r -> p k r", p=P))
                    ps = p1ps.tile([P, F1], f32)
                    for k in range(K0):
                        nc.tensor.matmul(
                            out=ps[:rows, :],
                            lhsT=xt[:, k * rows:(k + 1) * rows],
                            rhs=w1_sb[:, k * F1:(k + 1) * F1],
                            start=(k == 0), stop=(k == K0 - 1))
                    os = p1o.tile([P, F1], bf16)
                    nc.scalar.activation(out=os[:rows, :], in_=ps[:rows, :],
                                         func=mybir.ActivationFunctionType.Copy)
                    nc.sync.dma_start(out=ag1_in[m * P: m * P + rows, :],
                                      in_=os[:rows, :])

            nc.gpsimd.collective_compute(
                "AllGather", mybir.AluOpType.bypass, replica_groups=rg,
                ins=[ag1_in[:].opt()], outs=[ag1_out[:].opt()])

            # ------- aggregation helper (shared by phases 2 and 4) -------
            def aggregate(t, ag_out, F, pools_g, pool_s, issued_g, issued_s,
                          ps, tag):
                """Accumulate S.T @ G over tile t's chunks into psum ps."""
                first = True
                for h in range(NH):
                    g = h * ntiles + t
                    for j in range(offs[g], offs[g + 1]):
                        si = sg_of_chunk[j]
                        if si not in issued_g:
                            hh, w0, w1 = sg_plan[si]
                            nch = w1 - w0
                            J = nch * P
                            q = si % len(pools_g)
                            G = pools_g[q].tile(
                                [P, SG * F], bf16, tag=f"{tag}g{q}",
                                name=f"{tag}g{si}")
                            nc.gpsimd.dma_gather(
                                out_ap=G[:, :nch * F].rearrange(
                                    "p (c f) -> p c f", c=nch),
                                in_ap=ag_out[hh * half:(hh + 1) * half, :],
                                idxs_ap=idx_sb[:, w0 * 8: w0 * 8 + J // 16],
                                num_idxs=J, num_idxs_reg=J, elem_size=F,
                                queue_num=q)
                            issued_g[si] = (G, w0)
                        vi = sl_of_chunk[j]
                        if vi not in issued_s:
                            v0, v1 = sl_plan[vi]
                            S = pool_s.tile([P, SL * P], bf16, tag=f"{tag}s",
                                            name=f"{tag}s{vi}")
                            nc.sync.dma_start(
                                out=S[:, :(v1 - v0) * P],
                                in_=smat_d[:, v0 * P: v1 * P])
                            issued_s[vi] = (S, v0)
                        G, w0 = issued_g[si]
                        S, v0 = issued_s[vi]
                        lc = j - w0
                        ls = j - v0
                        nc.tensor.matmul(
                            out=ps[:], lhsT=S[:, ls * P:(ls + 1) * P],
                            rhs=G[:, lc * F:(lc + 1) * F],
                            start=first, stop=False)
                        first = False

            # ------- phase 2: aggregate layer 1 (node-major), then
            #         relu+bias and PE-transpose into h1T -------
            with (
                tc.tile_pool(name="p2g0", bufs=3) as p2g0,
                tc.tile_pool(name="p2g1", bufs=3) as p2g1,
                tc.tile_pool(name="p2g2", bufs=3) as p2g2,
                tc.tile_pool(name="p2g3", bufs=3) as p2g3,
                tc.tile_pool(name="p2s", bufs=3) as p2s,
                tc.tile_pool(name="p2h", bufs=3) as p2h,
                tc.tile_pool(name="p2ps", bufs=3, space="PSUM") as p2ps,
                tc.tile_pool(name="p2pt", bufs=3, space="PSUM") as p2pt,
            ):
                issued_g, issued_s = {}, {}
                p2gs = [p2g0, p2g1, p2g2, p2g3]
                for t in range(ntiles):
                    ps = p2ps.tile([P, F1], f32, tag="ps", name=f"ps_{t}")
                    aggregate(t, ag1_out, F1, p2gs, p2s, issued_g, issued_s,
                              ps, "A")
                    # += ones^T @ b1 (adds b1 to every row)
                    nc.tensor.matmul(out=ps[:], lhsT=ones_col[:],
                                     rhs=b1_row[:], start=False, stop=True)
                    hm = p2h.tile([P, F1], bf16, tag="hm")
                    nc.scalar.activation(
                        out=hm[:], in_=ps[:],
                        func=mybir.ActivationFunctionType.Relu)
                    for h in range(H1):
                        pt = p2pt.tile([P, P], bf16, tag="pt")
                        nc.tensor.transpose(
                            out=pt[:], in_=hm[:, h * P:(h + 1) * P],
                            identity=ident[:])
                        nc.vector.tensor_copy(
                            out=h1T[:, h * RP + t * P: h * RP + (t + 1) * P],
                            in_=pt[:])

            # ---------------- phase 3: hw2 = h1 @ W2 ----------------
            with (
                tc.tile_pool(name="p3o", bufs=3) as p3o,
                tc.tile_pool(name="p3ps", bufs=2, space="PSUM") as p3ps,
            ):
                for m in range(ntiles):
                    rows = last_rows if m == ntiles - 1 else P
                    ps = p3ps.tile([P, F2], f32)
                    for k in range(K2):
                        nc.tensor.matmul(
                            out=ps[:rows, :],
                            lhsT=h1T[:, k * RP + m * P: k * RP + m * P + rows],
                            rhs=w2_sb[:, k * F2:(k + 1) * F2],
                            start=(k == 0), stop=(k == K2 - 1))
                    os = p3o.tile([P, F2], bf16)
                    nc.scalar.activation(out=os[:rows, :], in_=ps[:rows, :],
                                         func=mybir.ActivationFunctionType.Copy)
                    nc.sync.dma_start(out=ag2_in[m * P: m * P + rows, :],
                                      in_=os[:rows, :])

            nc.gpsimd.collective_compute(
                "AllGather", mybir.AluOpType.bypass, replica_groups=rg,
                ins=[ag2_in[:].opt()], outs=[ag2_out[:].opt()])

            # ------- phase 4: aggregate layer 2, node-major out -------
            with (
                tc.tile_pool(name="p4g0", bufs=3) as p4g0,
                tc.tile_pool(name="p4g1", bufs=3) as p4g1,
                tc.tile_pool(name="p4g2", bufs=3) as p4g2,
                tc.tile_pool(name="p4g3", bufs=3) as p4g3,
                tc.tile_pool(name="p4s", bufs=3) as p4s,
                tc.tile_pool(name="p4o", bufs=3) as p4o,
                tc.tile_pool(name="p4ps", bufs=3, space="PSUM") as p4ps,
            ):
                issued_g, issued_s = {}, {}
                p4gs = [p4g0, p4g1, p4g2, p4g3]
                for t in range(ntiles):
                    rows = last_rows if t == ntiles - 1 else P
                    ps = p4ps.tile([P, F2], f32)
                    aggregate(t, ag2_out, F2, p4gs, p4s, issued_g, issued_s,
                              ps, "B")
                    # bias: += ones^T @ b2  (rank-1, adds b2 to every row)
                    nc.tensor.matmul(out=ps[:], lhsT=ones_col[:],
                                     rhs=b2_row[:], start=False, stop=True)
                    os = p4o.tile([P, F2], f32)
                    nc.scalar.activation(out=os[:rows, :], in_=ps[:rows, :],
                                         func=mybir.ActivationFunctionType.Copy)
                    nc.sync.dma_start(out=out_d[t * P: t * P + rows, :],
                                      in_=os[:rows, :])

    nc.compile()
    return nc


# ----------------------------------------------------------------------------
# Public entry point
# ----------------------------------------------------------------------------

LAST_EXEC_NS = None
LAST_RESULTS = None


def kernel(x, edge_index, W1, b1, W2, b2, _trace=False, _tmpdir=None):
    global LAST_EXEC_NS, LAST_RESULTS
    x = np.asarray(x, np.float32)
    edge_index = np.asarray(edge_index)
    W1 = np.asarray(W1, np.float32)
    b1 = np.asarray(b1, np.float32)
    W2 = np.asarray(W2, np.float32)
    b2 = np.asarray(b2, np.float32)
    N, F0 = x.shape
    F1 = W1.shape[1]
    F2 = W2.shape[1]

    packed, chunks, offs, R, ntiles, half = _preprocess(x, edge_index, NCORES)
    nc = build_nc(N, R, ntiles, chunks, offs, half, F0, F1, F2, NCORES)

    W1b = W1.astype(BF16)
    b1b = b1.astype(BF16)
    W2b = W2.astype(BF16)
    b2b = b2.astype(BF16)
    in_maps = []
    for c in range(NCORES):
        i_a, s_a = packed[c]
        xT_c = np.ascontiguousarray(x[c * R:(c + 1) * R].T.astype(BF16))
        in_maps.append({
            "xT": xT_c, "idx16": i_a, "smat": s_a,
            "W1": W1b, "b1": b1b, "W2": W2b, "b2": b2b,
        })

    res = bass_utils.run_bass_kernel_spmd(
        nc, in_maps, core_ids=list(range(NCORES)), trace=_trace,
        tmpdir=_tmpdir)
    LAST_EXEC_NS = res.exec_time_ns
    LAST_RESULTS = res
    out = np.concatenate([res.results[c]["out"] for c in range(NCORES)], axis=0)
    return out.astype(np.float32)


# revision 27
# speedup vs baseline: 1.2549x; 1.2549x over previous
"""2-layer GCN (PyG GCNConv style) on 8 Trainium2 NeuronCores.

Strategy (graph/node parallel, per sharding hint):
  - Nodes are range-sharded across 8 cores (R = N/8 rows each).
  - Host packs each core's incoming edges (incl. self-loops) into 128-edge
    chunks grouped by (source-node half, destination tile); pad slots get
    norm=0. Host also prebuilds the per-chunk selection matrices
    S[e, d] = (dst_local[e]==d) * norm[e] as one bf16 array ("smat"),
    consumed by both layers -- no per-chunk DVE work on device.
  - All PE-path data is bf16 (fp32 matmul is 4 cycles/row vs 1 for bf16;
    bf16 also halves AllGather wire bytes and gather descriptors).
  - Row gathers use the DMAGatherAnt custom instruction, batched up to
    SG=7 chunks (896 descriptors) per instruction: the SWDGE ring holds
    1024 descriptors (hard ucode limit, verified by probe) and descgen
    costs ~1us fixed per instruction. Indices are int16, so the gather
    source is split into NH=2 node-range halves (25000 rows < 32768).
  - Device per core:
      phase 1: xw1 = x_c @ W1 (own rows)           -> AllGather full xw1
      phase 2: stream (gather G rows | load S tiles), PE-matmul-
               accumulate S.T @ G per dst tile, ReLU+bias, PE-transpose
               into h1T (bf16, kept in SBUF)
      phase 3: hw2 = h1 @ W2 (uses h1T as lhsT)    -> AllGather full hw2
      phase 4: same aggregation on hw2, bias via rank-1 ones@b2 matmul,
               fp32 DMA to output.
  - Host concatenates the 8 row-shards.
"""

import sys

for p in ("/opt/trn_rl_repo",):
    if p not in sys.path:
        sys.path.insert(0, p)

import numpy as np
import ml_dtypes

import concourse.bacc as bacc
import concourse.mybir as mybir
import concourse.tile as tile
from concourse import bass_utils
from concourse.masks import make_identity

P = 128
NCORES = 8
NH = 2   # node-range halves for int16 gather indices
SG = 7   # max chunks per dma_gather (896 descs < 1024-desc SWDGE ring;
         # probe-measured best gather throughput at 4 SWDGE queues)
SL = 28  # chunks per S-matrix load (big contiguous HWDGE loads keep the
         # Sync engine off the critical path)
BF16 = ml_dtypes.bfloat16


# ----------------------------------------------------------------------------
# Host-side preprocessing
# ----------------------------------------------------------------------------

def _preprocess(x, edge_index, n_cores):
    """Packs per-core edges grouped by (src half, dst tile).

    Returns per-core (idx16, smat) + chunk table.
      idx16: [128, total*8] int16, dma_gather layout (16-partition
             wrapped, replicated 8x down the partition dim)
      smat:  [128, total*128] bf16, chunk j's S matrix in cols
             [j*128, (j+1)*128)
    """
    N = x.shape[0]
    R = N // n_cores
    assert R * n_cores == N
    ntiles = (R + P - 1) // P
    half = N // NH
    assert half * NH == N and half <= 32767

    src = edge_index[0].astype(np.int64)
    dst = edge_index[1].astype(np.int64)
    # degree includes the self-loop, but loops are NOT packed as edges:
    # they are applied on-device as diag(dis^2) against the core's own
    # (already local) feature rows -- no gather descriptors needed.
    deg = np.bincount(dst, minlength=N).astype(np.float32) + 1.0
    dis = 1.0 / np.sqrt(deg)
    norm = (dis[src] * dis[dst]).astype(np.float32)

    core_id = dst // R
    dloc = dst - core_id * R
    tl = dloc // P
    dstl = (dloc - tl * P).astype(np.int64)
    hf = src // half  # source half of each edge
    grp = hf * ntiles + tl  # per-core group id (half-major, tile-minor)

    ngrp = NH * ntiles
    counts = np.bincount(core_id * ngrp + grp,
                         minlength=n_cores * ngrp).reshape(n_cores, ngrp)
    # per-group chunk count = max over cores (SPMD program is shared)
    chunks = np.ceil(counts.max(axis=0) / P).astype(np.int64)  # [ngrp]
    total = int(chunks.sum())
    offs = np.concatenate([[0], np.cumsum(chunks)])  # chunk offsets per group

    packed = []
    for c in range(n_cores):
        m = core_id == c
        s_c = src[m].astype(np.int32)
        g_c = grp[m]
        d_c = dstl[m]
        n_c = norm[m]
        h_c = hf[m]
        order = np.argsort(g_c, kind="stable")
        s_c, g_c, d_c, n_c, h_c = (s_c[order], g_c[order], d_c[order],
                                   n_c[order], h_c[order])
        cnt = np.bincount(g_c, minlength=ngrp)
        starts = np.cumsum(cnt) - cnt
        pos = np.arange(len(g_c)) - np.repeat(starts, cnt)
        slots = offs[g_c] * P + pos  # slot s = chunk*128 + partition
        A_src = np.zeros(total * P, np.int32)  # half-local source idx
        A_src[slots] = s_c - h_c.astype(np.int32) * half

        # idx16: linear j = chunk*128+p -> [16, J/16] wrapped, replicated
        # 8x down partitions
        idx = A_src.astype(np.int16).reshape(total * P // 16, 16).T
        idx16 = np.ascontiguousarray(np.tile(idx, (8, 1)))  # [128, total*8]

        # smat: [128, (total+ntiles)*128]: S_j at cols j*128..(j+1)*128;
        # the tail ntiles blocks hold the self-loop diag(dis^2) per tile
        smat = np.zeros((P, (total + ntiles) * P), np.float32)
        pj = slots % P           # partition (edge slot in chunk)
        cj = slots // P          # global chunk
        smat[pj, cj * P + d_c] = n_c
        own = dis[c * R:(c + 1) * R] ** 2  # [R]
        rows_pad = np.zeros(ntiles * P, np.float32)
        rows_pad[:R] = own
        pp = np.arange(ntiles * P)
        smat[pp % P, (total + pp // P) * P + pp % P] = rows_pad
        packed.append((idx16, smat.astype(BF16)))
    return packed, chunks.tolist(), offs.tolist(), R, ntiles, half


# ----------------------------------------------------------------------------
# Device kernel builder
# ----------------------------------------------------------------------------

def build_nc(N, R, ntiles, chunks, offs, half, F0, F1, F2, n_cores,
             use_b1=True, use_b2=True):
    """Build the SPMD Bass program. All dims: F0,F1,F2 multiples of 128."""
    f32 = mybir.dt.float32
    bf16 = mybir.dt.bfloat16
    i16 = mybir.dt.int16
    K0 = F0 // P       # k-tiles in layer-1 matmul
    H1 = F1 // P       # 128-wide halves of F1
    K2 = F1 // P       # k-tiles in layer-2 matmul (= H1)
    assert F2 <= 512 and F2 % P == 0
    last_rows = R - (ntiles - 1) * P  # rows in the final (possibly partial) tile
    RP = ntiles * P    # padded row count
    total = int(sum(chunks))
    S16 = total * P // 16  # idx16 columns

    # super-gather plan: windows of <=SG chunks, never crossing a half
    # boundary (each half reads a different in_ap slice)
    sg_plan = []           # (h, w0, w1) in global chunk coords
    sg_of_chunk = {}       # chunk j -> sg index
    for h in range(NH):
        lo = offs[h * ntiles]
        hi = offs[(h + 1) * ntiles]
        w = lo
        while w < hi:
            w1 = min(w + SG, hi)
            for j in range(w, w1):
                sg_of_chunk[j] = len(sg_plan)
            sg_plan.append((h, w, w1))
            w = w1
    # S-load plan: big contiguous windows over the whole chunk range
    sl_plan = [(w, min(w + SL, total)) for w in range(0, total, SL)]
    sl_of_chunk = {}
    for i, (w0, w1) in enumerate(sl_plan):
        for j in range(w0, w1):
            sl_of_chunk[j] = i

    nc = bacc.Bacc("TRN2", target_bir_lowering=False, debug=False,
                   num_devices=n_cores, num_swdge_queues=4)

    xT = nc.dram_tensor("xT", [F0, R], bf16, kind="ExternalInput").ap()
    idx_d = nc.dram_tensor("idx16", [P, S16], i16, kind="ExternalInput").ap()
    smat_d = nc.dram_tensor("smat", [P, (total + ntiles) * P], bf16,
                            kind="ExternalInput").ap()
    W1_d = nc.dram_tensor("W1", [F0, F1], bf16, kind="ExternalInput").ap()
    b1_d = nc.dram_tensor("b1", [F1], bf16, kind="ExternalInput").ap()
    W2_d = nc.dram_tensor("W2", [F1, F2], bf16, kind="ExternalInput").ap()
    b2_d = nc.dram_tensor("b2", [F2], bf16, kind="ExternalInput").ap()
    out_d = nc.dram_tensor("out", [R, F2], f32, kind="ExternalOutput").ap()

    rg = [list(range(n_cores))]

    with tile.TileContext(nc) as tc:
        with (
            tc.tile_pool(name="dram", bufs=1, space="DRAM") as dram,
            tc.tile_pool(name="const", bufs=1) as const,
        ):
            ag1_in = dram.tile([R, F1], bf16)
            ag1_out = dram.tile([N, F1], bf16, addr_space="Shared")
            ag2_in = dram.tile([R, F2], bf16)
            ag2_out = dram.tile([N, F2], bf16, addr_space="Shared")

            # constants (single DMAs with 3D APs to keep consumer wait
            # counts low -- walrus limits sync-waits per instruction)
            w1_sb = const.tile([P, K0 * F1], bf16)
            nc.sync.dma_start(
                out=w1_sb[:].rearrange("p (k f) -> p k f", k=K0),
                in_=W1_d.rearrange("(k p) f -> p k f", p=P))
            w2_sb = const.tile([P, K2 * F2], bf16)
            nc.sync.dma_start(
                out=w2_sb[:].rearrange("p (k f) -> p k f", k=K2),
                in_=W2_d.rearrange("(k p) f -> p k f", p=P))
            b1_row = const.tile([1, F1], bf16)
            nc.sync.dma_start(out=b1_row[:, :], in_=b1_d[None, :])
            b2_row = const.tile([1, F2], bf16)
            nc.sync.dma_start(out=b2_row[:, :], in_=b2_d[None, :])
            ones_col = const.tile([1, P], bf16)
            nc.vector.memset(ones_col[:], 1.0)
            ident = const.tile([P, P], bf16)
            make_identity(nc, ident[:])

            idx_sb = const.tile([P, S16], i16)
            nc.sync.dma_start(out=idx_sb[:], in_=idx_d[:])

            h1T = const.tile([P, H1 * RP], bf16)  # h1 transposed, H1 row-blocks

            # ---------------- phase 1: xw1 = x_c @ W1 ----------------
            with (
                tc.tile_pool(name="p1x", bufs=3) as p1x,
                tc.tile_pool(name="p1o", bufs=3) as p1o,
                tc.tile_pool(name="p1ps", bufs=2, space="PSUM") as p1ps,
            ):
                for m in range(ntiles):
                    rows = last_rows if m == ntiles - 1 else P
                    xt = p1x.tile([P, K0 * P], bf16, tag="xt")
                    nc.sync.dma_start(
                        out=xt[:, : K0 * rows].rearrange(
                            "p (k r) -> p k r", k=K0),
                        in_=xT[:, m * P: m * P + rows].rearrange(
                            "(k p) r -> p k r", p=P))
                    ps = p1ps.tile([P, F1], f32)
                    for k in range(K0):
                        nc.tensor.matmul(
                            out=ps[:rows, :],
                            lhsT=xt[:, k * rows:(k + 1) * rows],
                            rhs=w1_sb[:, k * F1:(k + 1) * F1],
                            start=(k == 0), stop=(k == K0 - 1))
                    os = p1o.tile([P, F1], bf16)
                    nc.scalar.activation(out=os[:rows, :], in_=ps[:rows, :],
                                         func=mybir.ActivationFunctionType.Copy)
                    nc.sync.dma_start(out=ag1_in[m * P: m * P + rows, :],
                                      in_=os[:rows, :])

            nc.gpsimd.collective_compute(
                "AllGather", mybir.AluOpType.bypass, replica_groups=rg,
                ins=[ag1_in[:].opt()], outs=[ag1_out[:].opt()])

            # ------- aggregation helper (shared by phases 2 and 4) -------
            def aggregate(t, ag_out, F, pools_g, pool_s, issued_g, issued_s,
                          ps, tag):
                """Accumulate S.T @ G over tile t's chunks into psum ps."""
                first = True
                for h in range(NH):
                    g = h * ntiles + t
                    for j in range(offs[g], offs[g + 1]):
                        si = sg_of_chunk[j]
                        if si not in issued_g:
                            hh, w0, w1 = sg_plan[si]
                            nch = w1 - w0
                            J = nch * P
                            q = si % len(pools_g)
                            G = pools_g[q].tile(
                                [P, SG * F], bf16, tag=f"{tag}g{q}",
                                name=f"{tag}g{si}")
                            nc.gpsimd.dma_gather(
                                out_ap=G[:, :nch * F].rearrange(
                                    "p (c f) -> p c f", c=nch),
                                in_ap=ag_out[hh * half:(hh + 1) * half, :],
                                idxs_ap=idx_sb[:, w0 * 8: w0 * 8 + J // 16],
                                num_idxs=J, num_idxs_reg=J, elem_size=F,
                                queue_num=q)
                            issued_g[si] = (G, w0)
                        vi = sl_of_chunk[j]
                        if vi not in issued_s:
                            v0, v1 = sl_plan[vi]
                            S = pool_s.tile([P, SL * P], bf16, tag=f"{tag}s",
                                            name=f"{tag}s{vi}")
                            nc.sync.dma_start(
                                out=S[:, :(v1 - v0) * P],
                                in_=smat_d[:, v0 * P: v1 * P])
                            issued_s[vi] = (S, v0)
                        G, w0 = issued_g[si]
                        S, v0 = issued_s[vi]
                        lc = j - w0
                        ls = j - v0
                        nc.tensor.matmul(
                            out=ps[:], lhsT=S[:, ls * P:(ls + 1) * P],
                            rhs=G[:, lc * F:(lc + 1) * F],
                            start=first, stop=False)
                        first = False
                return first

            # ------- phase 2: aggregate layer 1 (node-major), then
            #         relu+bias, PE-transpose into h1T, and (interleaved
            #         phase 3) hw2 tile = h1 @ W2 -> ag2_in -------
            with (
                tc.tile_pool(name="p2g0", bufs=5) as p2g0,
                tc.tile_pool(name="p2g1", bufs=5) as p2g1,
                tc.tile_pool(name="p2g2", bufs=5) as p2g2,
                tc.tile_pool(name="p2g3", bufs=5) as p2g3,
                tc.tile_pool(name="p2s", bufs=3) as p2s,
                tc.tile_pool(name="p2lp", bufs=3) as p2lp,
                tc.tile_pool(name="p2gl", bufs=3) as p2gl,
                tc.tile_pool(name="p2h", bufs=3) as p2h,
                tc.tile_pool(name="p3o", bufs=3) as p3o,
                tc.tile_pool(name="p2ps", bufs=3, space="PSUM") as p2ps,
                tc.tile_pool(name="p2pt", bufs=3, space="PSUM") as p2pt,
                tc.tile_pool(name="p3ps", bufs=2, space="PSUM") as p3ps,
            ):
                issued_g, issued_s = {}, {}
                p2gs = [p2g0, p2g1, p2g2, p2g3]
                for t in range(ntiles):
                    rows = last_rows if t == ntiles - 1 else P
                    ps = p2ps.tile([P, F1], f32, tag="ps", name=f"ps_{t}")
                    first = aggregate(t, ag1_out, F1, p2gs, p2s, issued_g,
                                      issued_s, ps, "A")
                    # self-loops: += diag(dis^2) @ xw1_own_tile (local rows)
                    slp = p2lp.tile([P, P], bf16, tag="slp")
                    nc.sync.dma_start(
                        out=slp[:], in_=smat_d[:, (total + t) * P:
                                               (total + t + 1) * P])
                    glp = p2gl.tile([P, F1], bf16, tag="glp")
                    nc.sync.dma_start(out=glp[:rows, :],
                                      in_=ag1_in[t * P: t * P + rows, :])
                    nc.tensor.matmul(out=ps[:], lhsT=slp[:rows, :],
                                     rhs=glp[:rows, :],
                                     start=first, stop=not use_b1)
                    if use_b1:
                        # += ones^T @ b1 (adds b1 to every row)
                        nc.tensor.matmul(out=ps[:], lhsT=ones_col[:],
                                         rhs=b1_row[:], start=False, stop=True)
                    hm = p2h.tile([P, F1], bf16, tag="hm")
                    nc.scalar.activation(
                        out=hm[:], in_=ps[:],
                        func=mybir.ActivationFunctionType.Relu)
                    for h in range(H1):
                        pt = p2pt.tile([P, P], bf16, tag="pt")
                        nc.tensor.transpose(
                            out=pt[:], in_=hm[:, h * P:(h + 1) * P],
                            identity=ident[:])
                        nc.vector.tensor_copy(
                            out=h1T[:, h * RP + t * P: h * RP + (t + 1) * P],
                            in_=pt[:])
                    # interleaved phase 3 for this tile
                    ps3 = p3ps.tile([P, F2], f32, tag="ps3")
                    for k in range(K2):
                        nc.tensor.matmul(
                            out=ps3[:rows, :],
                            lhsT=h1T[:, k * RP + t * P: k * RP + t * P + rows],
                            rhs=w2_sb[:, k * F2:(k + 1) * F2],
                            start=(k == 0), stop=(k == K2 - 1))
                    os = p3o.tile([P, F2], bf16, tag="os3")
                    nc.scalar.activation(out=os[:rows, :], in_=ps3[:rows, :],
                                         func=mybir.ActivationFunctionType.Copy)
                    nc.sync.dma_start(out=ag2_in[t * P: t * P + rows, :],
                                      in_=os[:rows, :])

            nc.gpsimd.collective_compute(
                "AllGather", mybir.AluOpType.bypass, replica_groups=rg,
                ins=[ag2_in[:].opt()], outs=[ag2_out[:].opt()])

            # ------- phase 4: aggregate layer 2, node-major out -------
            with (
                tc.tile_pool(name="p4g0", bufs=5) as p4g0,
                tc.tile_pool(name="p4g1", bufs=5) as p4g1,
                tc.tile_pool(name="p4g2", bufs=5) as p4g2,
                tc.tile_pool(name="p4g3", bufs=5) as p4g3,
                tc.tile_pool(name="p4s", bufs=3) as p4s,
                tc.tile_pool(name="p4lp", bufs=3) as p4lp,
                tc.tile_pool(name="p4gl", bufs=3) as p4gl,
                tc.tile_pool(name="p4o", bufs=3) as p4o,
                tc.tile_pool(name="p4ps", bufs=3, space="PSUM") as p4ps,
            ):
                issued_g, issued_s = {}, {}
                p4gs = [p4g0, p4g1, p4g2, p4g3]
                for t in range(ntiles):
                    rows = last_rows if t == ntiles - 1 else P
                    ps = p4ps.tile([P, F2], f32)
                    first = aggregate(t, ag2_out, F2, p4gs, p4s, issued_g,
                                      issued_s, ps, "B")
                    # self-loops: += diag(dis^2) @ hw2_own_tile (local rows)
                    slp = p4lp.tile([P, P], bf16, tag="slp4")
                    nc.sync.dma_start(
                        out=slp[:], in_=smat_d[:, (total + t) * P:
                                               (total + t + 1) * P])
                    glp = p4gl.tile([P, F2], bf16, tag="glp4")
                    nc.sync.dma_start(out=glp[:rows, :],
                                      in_=ag2_in[t * P: t * P + rows, :])
                    nc.tensor.matmul(out=ps[:], lhsT=slp[:rows, :],
                                     rhs=glp[:rows, :],
                                     start=first, stop=not use_b2)
                    if use_b2:
                        # bias: += ones^T @ b2 (rank-1, adds b2 to every row)
                        nc.tensor.matmul(out=ps[:], lhsT=ones_col[:],
                                         rhs=b2_row[:], start=False, stop=True)
                    os = p4o.tile([P, F2], f32)
                    nc.scalar.activation(out=os[:rows, :], in_=ps[:rows, :],
                                         func=mybir.ActivationFunctionType.Copy)
                    nc.sync.dma_start(out=out_d[t * P: t * P + rows, :],
                                      in_=os[:rows, :])

    nc.compile()
    return nc


# ----------------------------------------------------------------------------
# Public entry point
# ----------------------------------------------------------------------------

LAST_EXEC_NS = None
LAST_RESULTS = None


def kernel(x, edge_index, W1, b1, W2, b2, _trace=False, _tmpdir=None):
    global LAST_EXEC_NS, LAST_RESULTS
    x = np.asarray(x, np.float32)
    edge_index = np.asarray(edge_index)
    W1 = np.asarray(W1, np.float32)
    b1 = np.asarray(b1, np.float32)
    W2 = np.asarray(W2, np.float32)
    b2 = np.asarray(b2, np.float32)
    N, F0 = x.shape
    F1 = W1.shape[1]
    F2 = W2.shape[1]

    packed, chunks, offs, R, ntiles, half = _preprocess(x, edge_index, NCORES)
    nc = build_nc(N, R, ntiles, chunks, offs, half, F0, F1, F2, NCORES,
                  use_b1=bool(np.any(b1)), use_b2=bool(np.any(b2)))

    W1b = W1.astype(BF16)
    b1b = b1.astype(BF16)
    W2b = W2.astype(BF16)
    b2b = b2.astype(BF16)
    in_maps = []
    for c in range(NCORES):
        i_a, s_a = packed[c]
        xT_c = np.ascontiguousarray(x[c * R:(c + 1) * R].T.astype(BF16))
        in_maps.append({
            "xT": xT_c, "idx16": i_a, "smat": s_a,
            "W1": W1b, "b1": b1b, "W2": W2b, "b2": b2b,
        })

    res = bass_utils.run_bass_kernel_spmd(
        nc, in_maps, core_ids=list(range(NCORES)), trace=_trace,
        tmpdir=_tmpdir)
    LAST_EXEC_NS = res.exec_time_ns
    LAST_RESULTS = res
    out = np.concatenate([res.results[c]["out"] for c in range(NCORES)], axis=0)
    return out.astype(np.float32)


# revision 32
# speedup vs baseline: 1.3088x; 1.0430x over previous
"""2-layer GCN (PyG GCNConv style) on 8 Trainium2 NeuronCores.

Strategy (graph/node parallel, per sharding hint):
  - Nodes are range-sharded across 8 cores (R = N/8 rows each).
  - Host packs each core's incoming edges (incl. self-loops) into 128-edge
    chunks grouped by (source-node half, destination tile); pad slots get
    norm=0. Host also prebuilds the per-chunk selection matrices
    S[e, d] = (dst_local[e]==d) * norm[e] as one bf16 array ("smat"),
    consumed by both layers -- no per-chunk DVE work on device.
  - All PE-path data is bf16 (fp32 matmul is 4 cycles/row vs 1 for bf16;
    bf16 also halves AllGather wire bytes and gather descriptors).
  - Row gathers use the DMAGatherAnt custom instruction, batched to SG
    chunks per instruction and round-robined over 4 SWDGE queues (Q7
    descgen costs ~1us fixed + ~8ns/descriptor per queue; the ring holds
    1024 descriptors -- a hard ucode limit, verified by probe). Indices
    are int16, so the gather source is split into NH=2 node-range halves
    (25000 rows < 32768).
  - Device per core:
      phase 1: xw1 = x_c @ W1 (own rows)           -> AllGather full xw1
      phase 2: stream (gather G rows | load S tiles), PE-matmul-
               accumulate S.T @ G per dst tile, ReLU+bias, PE-transpose
               into h1T (bf16, kept in SBUF)
      phase 3: hw2 = h1 @ W2 (uses h1T as lhsT)    -> AllGather full hw2
      phase 4: same aggregation on hw2, bias via rank-1 ones@b2 matmul,
               fp32 DMA to output.
  - Host concatenates the 8 row-shards.
"""

import sys

for p in ("/opt/trn_rl_repo",):
    if p not in sys.path:
        sys.path.insert(0, p)

import numpy as np
import ml_dtypes

import concourse.bacc as bacc
import concourse.mybir as mybir
import concourse.tile as tile
from concourse import bass_utils
from concourse.masks import make_identity

P = 128
NCORES = 8
NH = 2   # node-range halves for int16 gather indices
SG = 4   # chunks per dma_gather (512 descs; 2 windows fit the 1024-desc
         # SWDGE ring so per-queue descgen overlaps the previous drain)
SL = 28  # chunks per S-matrix load (big contiguous HWDGE loads keep the
         # Sync engine off the critical path)
BF16 = ml_dtypes.bfloat16


# ----------------------------------------------------------------------------
# Host-side preprocessing
# ----------------------------------------------------------------------------

def _preprocess(x, edge_index, n_cores):
    """Packs per-core edges grouped by (src half, dst tile).

    Returns per-core (idx16, smat) + chunk table.
      idx16: [128, total*8] int16, dma_gather layout (16-partition
             wrapped, replicated 8x down the partition dim)
      smat:  [128, total*128] bf16, chunk j's S matrix in cols
             [j*128, (j+1)*128)
    """
    N = x.shape[0]
    R = N // n_cores
    assert R * n_cores == N
    ntiles = (R + P - 1) // P
    half = N // NH
    assert half * NH == N and half <= 32767

    src = edge_index[0].astype(np.int64)
    dst = edge_index[1].astype(np.int64)
    # degree includes the self-loop, but loops are NOT packed as edges:
    # they are applied on-device as diag(dis^2) against the core's own
    # (already local) feature rows -- no gather descriptors needed.
    deg = np.bincount(dst, minlength=N).astype(np.float32) + 1.0
    dis = 1.0 / np.sqrt(deg)
    norm = (dis[src] * dis[dst]).astype(np.float32)

    core_id = dst // R
    dloc = dst - core_id * R
    tl = dloc // P
    dstl = (dloc - tl * P).astype(np.int64)
    hf = src // half  # source half of each edge
    grp = hf * ntiles + tl  # per-core group id (half-major, tile-minor)

    ngrp = NH * ntiles
    counts = np.bincount(core_id * ngrp + grp,
                         minlength=n_cores * ngrp).reshape(n_cores, ngrp)
    # per-group chunk count = max over cores (SPMD program is shared)
    chunks = np.ceil(counts.max(axis=0) / P).astype(np.int64)  # [ngrp]
    total = int(chunks.sum())
    offs = np.concatenate([[0], np.cumsum(chunks)])  # chunk offsets per group

    packed = []
    for c in range(n_cores):
        m = core_id == c
        s_c = src[m].astype(np.int32)
        g_c = grp[m]
        d_c = dstl[m]
        n_c = norm[m]
        h_c = hf[m]
        order = np.argsort(g_c, kind="stable")
        s_c, g_c, d_c, n_c, h_c = (s_c[order], g_c[order], d_c[order],
                                   n_c[order], h_c[order])
        cnt = np.bincount(g_c, minlength=ngrp)
        starts = np.cumsum(cnt) - cnt
        pos = np.arange(len(g_c)) - np.repeat(starts, cnt)
        slots = offs[g_c] * P + pos  # slot s = chunk*128 + partition
        A_src = np.zeros(total * P, np.int32)  # half-local source idx
        A_src[slots] = s_c - h_c.astype(np.int32) * half

        # idx16: linear j = chunk*128+p -> [16, J/16] wrapped, replicated
        # 8x down partitions
        idx = A_src.astype(np.int16).reshape(total * P // 16, 16).T
        idx16 = np.ascontiguousarray(np.tile(idx, (8, 1)))  # [128, total*8]

        # smat: [128, (total+ntiles)*128]: S_j at cols j*128..(j+1)*128;
        # the tail ntiles blocks hold the self-loop diag(dis^2) per tile
        smat = np.zeros((P, (total + ntiles) * P), np.float32)
        pj = slots % P           # partition (edge slot in chunk)
        cj = slots // P          # global chunk
        smat[pj, cj * P + d_c] = n_c
        own = dis[c * R:(c + 1) * R] ** 2  # [R]
        rows_pad = np.zeros(ntiles * P, np.float32)
        rows_pad[:R] = own
        pp = np.arange(ntiles * P)
        smat[pp % P, (total + pp // P) * P + pp % P] = rows_pad
        packed.append((idx16, smat.astype(BF16)))
    return packed, chunks.tolist(), offs.tolist(), R, ntiles, half


# ----------------------------------------------------------------------------
# Device kernel builder
# ----------------------------------------------------------------------------

def build_nc(N, R, ntiles, chunks, offs, half, F0, F1, F2, n_cores,
             use_b1=True, use_b2=True):
    """Build the SPMD Bass program. All dims: F0,F1,F2 multiples of 128."""
    f32 = mybir.dt.float32
    bf16 = mybir.dt.bfloat16
    i16 = mybir.dt.int16
    K0 = F0 // P       # k-tiles in layer-1 matmul
    H1 = F1 // P       # 128-wide halves of F1
    K2 = F1 // P       # k-tiles in layer-2 matmul (= H1)
    assert F2 <= 512 and F2 % P == 0
    last_rows = R - (ntiles - 1) * P  # rows in the final (possibly partial) tile
    RP = ntiles * P    # padded row count
    total = int(sum(chunks))
    S16 = total * P // 16  # idx16 columns

    # super-gather plan: windows of <=SG chunks, never crossing a half
    # boundary (each half reads a different in_ap slice)
    sg_plan = []           # (h, w0, w1) in global chunk coords
    sg_of_chunk = {}       # chunk j -> sg index
    for h in range(NH):
        lo = offs[h * ntiles]
        hi = offs[(h + 1) * ntiles]
        w = lo
        while w < hi:
            w1 = min(w + SG, hi)
            for j in range(w, w1):
                sg_of_chunk[j] = len(sg_plan)
            sg_plan.append((h, w, w1))
            w = w1
    # S-load plan: big contiguous windows over the whole chunk range
    sl_plan = [(w, min(w + SL, total)) for w in range(0, total, SL)]
    sl_of_chunk = {}
    for i, (w0, w1) in enumerate(sl_plan):
        for j in range(w0, w1):
            sl_of_chunk[j] = i

    nc = bacc.Bacc("TRN2", target_bir_lowering=False, debug=False,
                   num_devices=n_cores, num_swdge_queues=4)

    xT = nc.dram_tensor("xT", [F0, R], bf16, kind="ExternalInput").ap()
    idx_d = nc.dram_tensor("idx16", [P, S16], i16, kind="ExternalInput").ap()
    smat_d = nc.dram_tensor("smat", [P, (total + ntiles) * P], bf16,
                            kind="ExternalInput").ap()
    W1_d = nc.dram_tensor("W1", [F0, F1], bf16, kind="ExternalInput").ap()
    b1_d = nc.dram_tensor("b1", [F1], bf16, kind="ExternalInput").ap()
    W2_d = nc.dram_tensor("W2", [F1, F2], bf16, kind="ExternalInput").ap()
    b2_d = nc.dram_tensor("b2", [F2], bf16, kind="ExternalInput").ap()
    out_d = nc.dram_tensor("out", [R, F2], f32, kind="ExternalOutput").ap()

    rg = [list(range(n_cores))]

    with tile.TileContext(nc) as tc:
        with (
            tc.tile_pool(name="dram", bufs=1, space="DRAM") as dram,
            tc.tile_pool(name="const", bufs=1) as const,
        ):
            ag1_in = dram.tile([R, F1], bf16)
            ag1_out = dram.tile([N, F1], bf16, addr_space="Shared")
            ag2_in = dram.tile([R, F2], bf16)
            ag2_out = dram.tile([N, F2], bf16, addr_space="Shared")

            # constants (single DMAs with 3D APs to keep consumer wait
            # counts low -- walrus limits sync-waits per instruction)
            w1_sb = const.tile([P, K0 * F1], bf16)
            nc.sync.dma_start(
                out=w1_sb[:].rearrange("p (k f) -> p k f", k=K0),
                in_=W1_d.rearrange("(k p) f -> p k f", p=P))
            w2_sb = const.tile([P, K2 * F2], bf16)
            nc.sync.dma_start(
                out=w2_sb[:].rearrange("p (k f) -> p k f", k=K2),
                in_=W2_d.rearrange("(k p) f -> p k f", p=P))
            b1_row = const.tile([1, F1], bf16)
            nc.sync.dma_start(out=b1_row[:, :], in_=b1_d[None, :])
            b2_row = const.tile([1, F2], bf16)
            nc.sync.dma_start(out=b2_row[:, :], in_=b2_d[None, :])
            ones_col = const.tile([1, P], bf16)
            nc.vector.memset(ones_col[:], 1.0)
            ident = const.tile([P, P], bf16)
            make_identity(nc, ident[:])

            idx_sb = const.tile([P, S16], i16)
            nc.sync.dma_start(out=idx_sb[:], in_=idx_d[:])

            h1T = const.tile([P, H1 * RP], bf16)  # h1 transposed, H1 row-blocks

            # ---------------- phase 1: xw1 = x_c @ W1 ----------------
            with (
                tc.tile_pool(name="p1x", bufs=3) as p1x,
                tc.tile_pool(name="p1o", bufs=3) as p1o,
                tc.tile_pool(name="p1ps", bufs=2, space="PSUM") as p1ps,
            ):
                for m in range(ntiles):
                    rows = last_rows if m == ntiles - 1 else P
                    xt = p1x.tile([P, K0 * P], bf16, tag="xt")
                    nc.sync.dma_start(
                        out=xt[:, : K0 * rows].rearrange(
                            "p (k r) -> p k r", k=K0),
                        in_=xT[:, m * P: m * P + rows].rearrange(
                            "(k p) r -> p k r", p=P))
                    ps = p1ps.tile([P, F1], f32)
                    for k in range(K0):
                        nc.tensor.matmul(
                            out=ps[:rows, :],
                            lhsT=xt[:, k * rows:(k + 1) * rows],
                            rhs=w1_sb[:, k * F1:(k + 1) * F1],
                            start=(k == 0), stop=(k == K0 - 1))
                    os = p1o.tile([P, F1], bf16)
                    nc.scalar.activation(out=os[:rows, :], in_=ps[:rows, :],
                                         func=mybir.ActivationFunctionType.Copy)
                    nc.sync.dma_start(out=ag1_in[m * P: m * P + rows, :],
                                      in_=os[:rows, :])

            nc.gpsimd.collective_compute(
                "AllGather", mybir.AluOpType.bypass, replica_groups=rg,
                ins=[ag1_in[:].opt()], outs=[ag1_out[:].opt()])

            # ------- aggregation helper (shared by phases 2 and 4) -------
            def aggregate(t, ag_out, F, pools_g, pool_s, issued_g, issued_s,
                          ps, tag):
                """Accumulate S.T @ G over tile t's chunks into psum ps."""
                first = True
                for h in range(NH):
                    g = h * ntiles + t
                    for j in range(offs[g], offs[g + 1]):
                        si = sg_of_chunk[j]
                        if si not in issued_g:
                            hh, w0, w1 = sg_plan[si]
                            nch = w1 - w0
                            J = nch * P
                            q = si % len(pools_g)
                            G = pools_g[q].tile(
                                [P, SG * F], bf16, tag=f"{tag}g{q}",
                                name=f"{tag}g{si}")
                            nc.gpsimd.dma_gather(
                                out_ap=G[:, :nch * F].rearrange(
                                    "p (c f) -> p c f", c=nch),
                                in_ap=ag_out[hh * half:(hh + 1) * half, :],
                                idxs_ap=idx_sb[:, w0 * 8: w0 * 8 + J // 16],
                                num_idxs=J, num_idxs_reg=J, elem_size=F,
                                queue_num=q)
                            issued_g[si] = (G, w0)
                        vi = sl_of_chunk[j]
                        if vi not in issued_s:
                            v0, v1 = sl_plan[vi]
                            S = pool_s.tile([P, SL * P], bf16, tag=f"{tag}s",
                                            name=f"{tag}s{vi}")
                            nc.sync.dma_start(
                                out=S[:, :(v1 - v0) * P],
                                in_=smat_d[:, v0 * P: v1 * P])
                            issued_s[vi] = (S, v0)
                        G, w0 = issued_g[si]
                        S, v0 = issued_s[vi]
                        lc = j - w0
                        ls = j - v0
                        nc.tensor.matmul(
                            out=ps[:], lhsT=S[:, ls * P:(ls + 1) * P],
                            rhs=G[:, lc * F:(lc + 1) * F],
                            start=first, stop=False)
                        first = False
                return first

            # ------- phase 2: aggregate layer 1 (node-major), then
            #         relu+bias, PE-transpose into h1T, and (interleaved
            #         phase 3) hw2 tile = h1 @ W2 -> ag2_in -------
            with (
                tc.tile_pool(name="p2g0", bufs=5) as p2g0,
                tc.tile_pool(name="p2g1", bufs=5) as p2g1,
                tc.tile_pool(name="p2g2", bufs=5) as p2g2,
                tc.tile_pool(name="p2g3", bufs=5) as p2g3,
                tc.tile_pool(name="p2s", bufs=3) as p2s,
                tc.tile_pool(name="p2lp", bufs=3) as p2lp,
                tc.tile_pool(name="p2gl", bufs=3) as p2gl,
                tc.tile_pool(name="p2h", bufs=3) as p2h,
                tc.tile_pool(name="p3o", bufs=3) as p3o,
                tc.tile_pool(name="p2ps", bufs=3, space="PSUM") as p2ps,
                tc.tile_pool(name="p2pt", bufs=3, space="PSUM") as p2pt,
                tc.tile_pool(name="p3ps", bufs=2, space="PSUM") as p3ps,
            ):
                issued_g, issued_s = {}, {}
                p2gs = [p2g0, p2g1, p2g2, p2g3]
                for t in range(ntiles):
                    rows = last_rows if t == ntiles - 1 else P
                    ps = p2ps.tile([P, F1], f32, tag="ps", name=f"ps_{t}")
                    first = aggregate(t, ag1_out, F1, p2gs, p2s, issued_g,
                                      issued_s, ps, "A")
                    # self-loops: += diag(dis^2) @ xw1_own_tile (local rows)
                    slp = p2lp.tile([P, P], bf16, tag="slp")
                    nc.sync.dma_start(
                        out=slp[:], in_=smat_d[:, (total + t) * P:
                                               (total + t + 1) * P])
                    glp = p2gl.tile([P, F1], bf16, tag="glp")
                    nc.sync.dma_start(out=glp[:rows, :],
                                      in_=ag1_in[t * P: t * P + rows, :])
                    nc.tensor.matmul(out=ps[:], lhsT=slp[:rows, :],
                                     rhs=glp[:rows, :],
                                     start=first, stop=not use_b1)
                    if use_b1:
                        # += ones^T @ b1 (adds b1 to every row)
                        nc.tensor.matmul(out=ps[:], lhsT=ones_col[:],
                                         rhs=b1_row[:], start=False, stop=True)
                    hm = p2h.tile([P, F1], bf16, tag="hm")
                    nc.scalar.activation(
                        out=hm[:], in_=ps[:],
                        func=mybir.ActivationFunctionType.Relu)
                    for h in range(H1):
                        pt = p2pt.tile([P, P], bf16, tag="pt")
                        nc.tensor.transpose(
                            out=pt[:], in_=hm[:, h * P:(h + 1) * P],
                            identity=ident[:])
                        nc.vector.tensor_copy(
                            out=h1T[:, h * RP + t * P: h * RP + (t + 1) * P],
                            in_=pt[:])
                    # interleaved phase 3 for this tile
                    ps3 = p3ps.tile([P, F2], f32, tag="ps3")
                    for k in range(K2):
                        nc.tensor.matmul(
                            out=ps3[:rows, :],
                            lhsT=h1T[:, k * RP + t * P: k * RP + t * P + rows],
                            rhs=w2_sb[:, k * F2:(k + 1) * F2],
                            start=(k == 0), stop=(k == K2 - 1))
                    os = p3o.tile([P, F2], bf16, tag="os3")
                    nc.scalar.activation(out=os[:rows, :], in_=ps3[:rows, :],
                                         func=mybir.ActivationFunctionType.Copy)
                    nc.sync.dma_start(out=ag2_in[t * P: t * P + rows, :],
                                      in_=os[:rows, :])

            nc.gpsimd.collective_compute(
                "AllGather", mybir.AluOpType.bypass, replica_groups=rg,
                ins=[ag2_in[:].opt()], outs=[ag2_out[:].opt()])

            # ------- phase 4: aggregate layer 2, node-major out -------
            with (
                tc.tile_pool(name="p4g0", bufs=5) as p4g0,
                tc.tile_pool(name="p4g1", bufs=5) as p4g1,
                tc.tile_pool(name="p4g2", bufs=5) as p4g2,
                tc.tile_pool(name="p4g3", bufs=5) as p4g3,
                tc.tile_pool(name="p4s", bufs=3) as p4s,
                tc.tile_pool(name="p4lp", bufs=3) as p4lp,
                tc.tile_pool(name="p4gl", bufs=3) as p4gl,
                tc.tile_pool(name="p4o", bufs=3) as p4o,
                tc.tile_pool(name="p4ps", bufs=3, space="PSUM") as p4ps,
            ):
                issued_g, issued_s = {}, {}
                p4gs = [p4g0, p4g1, p4g2, p4g3]
                for t in range(ntiles):
                    rows = last_rows if t == ntiles - 1 else P
                    ps = p4ps.tile([P, F2], f32)
                    first = aggregate(t, ag2_out, F2, p4gs, p4s, issued_g,
                                      issued_s, ps, "B")
                    # self-loops: += diag(dis^2) @ hw2_own_tile (local rows)
                    slp = p4lp.tile([P, P], bf16, tag="slp4")
                    nc.sync.dma_start(
                        out=slp[:], in_=smat_d[:, (total + t) * P:
                                               (total + t + 1) * P])
                    glp = p4gl.tile([P, F2], bf16, tag="glp4")
                    nc.sync.dma_start(out=glp[:rows, :],
                                      in_=ag2_in[t * P: t * P + rows, :])
                    nc.tensor.matmul(out=ps[:], lhsT=slp[:rows, :],
                                     rhs=glp[:rows, :],
                                     start=first, stop=not use_b2)
                    if use_b2:
                        # bias: += ones^T @ b2 (rank-1, adds b2 to every row)
                        nc.tensor.matmul(out=ps[:], lhsT=ones_col[:],
                                         rhs=b2_row[:], start=False, stop=True)
                    os = p4o.tile([P, F2], f32)
                    nc.scalar.activation(out=os[:rows, :], in_=ps[:rows, :],
                                         func=mybir.ActivationFunctionType.Copy)
                    nc.sync.dma_start(out=out_d[t * P: t * P + rows, :],
                                      in_=os[:rows, :])

    nc.compile()
    return nc


# ----------------------------------------------------------------------------
# Public entry point
# ----------------------------------------------------------------------------

LAST_EXEC_NS = None
LAST_RESULTS = None


def kernel(x, edge_index, W1, b1, W2, b2, _trace=False, _tmpdir=None):
    global LAST_EXEC_NS, LAST_RESULTS
    x = np.asarray(x, np.float32)
    edge_index = np.asarray(edge_index)
    W1 = np.asarray(W1, np.float32)
    b1 = np.asarray(b1, np.float32)
    W2 = np.asarray(W2, np.float32)
    b2 = np.asarray(b2, np.float32)
    N, F0 = x.shape
    F1 = W1.shape[1]
    F2 = W2.shape[1]

    packed, chunks, offs, R, ntiles, half = _preprocess(x, edge_index, NCORES)
    nc = build_nc(N, R, ntiles, chunks, offs, half, F0, F1, F2, NCORES,
                  use_b1=bool(np.any(b1)), use_b2=bool(np.any(b2)))

    W1b = W1.astype(BF16)
    b1b = b1.astype(BF16)
    W2b = W2.astype(BF16)
    b2b = b2.astype(BF16)
    in_maps = []
    for c in range(NCORES):
        i_a, s_a = packed[c]
        xT_c = np.ascontiguousarray(x[c * R:(c + 1) * R].T.astype(BF16))
        in_maps.append({
            "xT": xT_c, "idx16": i_a, "smat": s_a,
            "W1": W1b, "b1": b1b, "W2": W2b, "b2": b2b,
        })

    res = bass_utils.run_bass_kernel_spmd(
        nc, in_maps, core_ids=list(range(NCORES)), trace=_trace,
        tmpdir=_tmpdir)
    LAST_EXEC_NS = res.exec_time_ns
    LAST_RESULTS = res
    out = np.concatenate([res.results[c]["out"] for c in range(NCORES)], axis=0)
    return out.astype(np.float32)
